# revision 15
# baseline (speedup 1.0000x reference)
"""Trainium2 Bass kernel for ensemble Conv2d (VALID, 3x3).

Problem: out[e,b,j,y,x] = sum_{i,kh,kw} features[e,b,i,y+kh,x+kw] * weight[e,i,j,kh,kw] + bias[e,j]
  features: (4, 32, 64, 64, 64) f32, weight: (4, 64, 128, 3, 3) f32, bias: (4, 128) f32
  output:   (4, 32, 128, 62, 62) f32

Sharding: E*B = 128 images over 8 cores -> each core handles one ensemble
member e = core//2 and 16 of its 32 images. No cross-core communication.

Per-core kernel: implicit-GEMM convolution. For each image, each 3x3 tap
(kh,kw) is one matmul contraction over C_in=64:
  psum[j, (y,x)] += W[:,j,kh,kw].T @ X[:, y+kh, x+kw]
fp16 matmuls (1 col/cycle, fp32 PSUM accumulation; rel err ~3e-4). Two
images are processed concurrently on the two PE-array row halves
(tile_position (0,0) and (64,0)) so the K=64 contraction doesn't waste half
the array; their weight copies live on the matching partition halves.

PSUM evacuation (bias add + f32->fp16 downcast) alternates between the
vector and gpsimd engines via tensor_scalar_add - no scalar activation, so
no ACT_TABLE_LOAD on the critical path. Output is stored fp16 (half the
HBM write traffic; host upcasts) in 4-row-block merged stores.
"""

import ml_dtypes
import numpy as np

import concourse.bass as bass
import concourse.mybir as mybir
import concourse.tile as tile
from concourse import bacc
from concourse.alu_op_type import AluOpType
from concourse.bass_utils import run_bass_kernel_spmd

E, B, C_IN, H, W = 4, 32, 64, 64, 64
C_OUT, KH, KW = 128, 3, 3
HO, WO = H - KH + 1, W - KW + 1  # 62, 62
N_CORES = 8
B_PER_CORE = (E * B) // N_CORES  # 16
PAIRS = B_PER_CORE // 2  # 8
ROW_BLOCK = 8  # output rows per matmul block: N = 8*62 = 496 <= 512 (one PSUM bank)
ROW_BLOCKS = [(r * ROW_BLOCK, min(ROW_BLOCK, HO - r * ROW_BLOCK))
              for r in range((HO + ROW_BLOCK - 1) // ROW_BLOCK)]
# merge groups of 4 row blocks into single output stores
STORE_GROUPS = [(0, 4), (4, 4)]  # (first block idx, n blocks)
F32 = mybir.dt.float32
F32R = mybir.dt.float32r
BF16 = mybir.dt.bfloat16
FP16 = mybir.dt.float16

MM_DT = FP16
OUT_DT = FP16

_CACHE: dict = {}


def _build():
    nc = bacc.Bacc("TRN2", target_bir_lowering=False, debug=False,
                   num_devices=N_CORES)
    x_d = nc.dram_tensor("x", [B_PER_CORE * C_IN, H * W], MM_DT,
                         kind="ExternalInput").ap()
    w_d = nc.dram_tensor("w", [128, KH * KW * C_OUT], MM_DT,
                         kind="ExternalInput").ap()
    b_d = nc.dram_tensor("bias", [C_OUT, 1], F32, kind="ExternalInput").ap()
    y_d = nc.dram_tensor("y", [B_PER_CORE * C_OUT, HO * WO], OUT_DT,
                         kind="ExternalOutput").ap()

    with tile.TileContext(nc) as tc:
        with (
            tc.tile_pool(name="wpool", bufs=1) as wpool,
            tc.tile_pool(name="xpool", bufs=6) as xpool,
            tc.tile_pool(name="opool", bufs=6) as opool,
            tc.tile_pool(name="psum", bufs=8, space=bass.MemorySpace.PSUM) as psum,
        ):
            # Ramp-critical loads on the two HWDGE rings (the gpsimd ring is
            # SWDGE - ~3us slower to first byte, so it carries no DMAs at
            # all): weights on scalar, the first 10 input rows of pair 0
            # (exactly what the first row-block needs) alone on sync so no
            # other packets share its queue.
            # tap 0's weight columns land first (33KB) so the first
            # LDWEIGHTS fires ~2.5us earlier than waiting for all of w
            w_sb = wpool.tile([128, KH * KW * C_OUT], MM_DT)
            nc.scalar.dma_start(w_sb[:, 0:C_OUT], w_d[:, 0:C_OUT])
            nc.scalar.dma_start(w_sb[:, C_OUT:], w_d[:, C_OUT:])
            bias_sb = wpool.tile([C_OUT, 1], F32)
            nc.scalar.dma_start(bias_sb[:, :], b_d[:, :])

            x_tiles = [None] * PAIRS
            C0 = (ROW_BLOCK + KH - 1) * W  # 640 cols = input rows 0-9

            def issue_x_dma(p):
                # image pair p: image 2p on partitions 0-63, 2p+1 on 64-127.
                # Column-chunks so sub-tile deps let each row-block start as
                # soon as its chunk lands.
                x_sb = xpool.tile([128, H * W], MM_DT, tag="x")
                bounds = [0, 1024, 2048, 3072, H * W]
                if p == 0:
                    nc.sync.dma_start(x_sb[:, 0:C0], x_d[0:128, 0:C0])
                    bounds = [C0, 1408, 2176, 3072, H * W]
                for c0, c1 in zip(bounds[:-1], bounds[1:]):
                    nc.scalar.dma_start(x_sb[:, c0:c1],
                                        x_d[p * 128:(p + 1) * 128, c0:c1])
                x_tiles[p] = x_sb

            issue_x_dma(0)
            issue_x_dma(1)

            issue_x_dma(2)

            for p in range(PAIRS):
                if p + 3 < PAIRS:
                    issue_x_dma(p + 3)
                xv = x_tiles[p].rearrange("p (r c) -> p r c", c=W)
                # one fp16 output tile per (half, store-group): 4 blocks merged
                o_sb = {}
                for h in (0, 1):
                    for g, (g0, gn) in enumerate(STORE_GROUPS):
                        ncols = sum(nr * WO for (R, nr) in ROW_BLOCKS[g0:g0 + gn])
                        o_sb[(h, g)] = opool.tile([C_OUT, ncols], OUT_DT,
                                                  tag="o", name=f"o{p}_{h}_{g}")
                for bi, (R, nr) in enumerate(ROW_BLOCKS):
                    n_free = nr * WO
                    ps = [psum.tile([C_OUT, n_free], F32, tag="ps",
                                    name=f"ps{p}_{R}_{h}")
                          for h in range(2)]
                    for t in range(KH * KW):
                        kh, kw = divmod(t, KW)
                        for h in (0, 1):
                            rhs = xv[64 * h:64 * h + 64,
                                     R + kh:R + kh + nr, kw:kw + WO]
                            lhsT = w_sb[64 * h:64 * h + 64,
                                        t * C_OUT:(t + 1) * C_OUT]
                            nc.tensor.matmul(
                                ps[h][:, :],
                                lhsT,
                                rhs,
                                start=(t == 0),
                                stop=(t == KH * KW - 1),
                                tile_position=(64 * h, 0),
                            )
                    g = 0 if bi < STORE_GROUPS[0][1] else 1
                    goff = (R - ROW_BLOCKS[STORE_GROUPS[g][0]][0]) * WO
                    for h in (0, 1):
                        # evacuate psum (bias add + downcast): vector and
                        # scalar split the work so the tail's last evacs run
                        # in parallel (gpsimd can't read PSUM)
                        if h == 0:
                            nc.vector.tensor_scalar_add(
                                o_sb[(h, g)][:, goff:goff + n_free],
                                ps[h][:, :], bias_sb[:, :])
                        else:
                            nc.scalar.activation(
                                o_sb[(h, g)][:, goff:goff + n_free],
                                ps[h][:, :],
                                mybir.ActivationFunctionType.Identity,
                                bias=bias_sb[:, :])
                for h in (0, 1):
                    b_img = p * 2 + h
                    for g, (g0, gn) in enumerate(STORE_GROUPS):
                        c0 = ROW_BLOCKS[g0][0] * WO
                        ncols = o_sb[(h, g)].shape[1]
                        if p < PAIRS - 1:
                            nc.sync.dma_start(
                                y_d[b_img * C_OUT:(b_img + 1) * C_OUT,
                                    c0:c0 + ncols],
                                o_sb[(h, g)][:, :])
                        else:
                            # last pair: per-block stores so the final store
                            # begins as soon as its own evac lands
                            off = 0
                            for (R, nr) in ROW_BLOCKS[g0:g0 + gn]:
                                n = nr * WO
                                nc.sync.dma_start(
                                    y_d[b_img * C_OUT:(b_img + 1) * C_OUT,
                                        c0 + off:c0 + off + n],
                                    o_sb[(h, g)][:, off:off + n])
                                off += n
    nc.compile()
    return nc


def _get_nc():
    if "nc" not in _CACHE:
        _CACHE["nc"] = _build()
    return _CACHE["nc"]


def _make_in_maps(features, weight, bias):
    features = np.asarray(features, dtype=np.float32)
    weight = np.asarray(weight, dtype=np.float32)
    bias = np.asarray(bias, dtype=np.float32)
    in_maps = []
    for c in range(N_CORES):
        e, half = divmod(c, 2)
        b0 = half * B_PER_CORE
        x = np.ascontiguousarray(features[e, b0:b0 + B_PER_CORE]).reshape(
            B_PER_CORE * C_IN, H * W).astype(np.float16)
        # w[i, (kh*KW+kw)*C_OUT + j] = weight[e, i, j, kh, kw]; duplicated on
        # partitions 64-127 for the upper-row-half matmuls.
        wp = weight[e].transpose(0, 2, 3, 1).reshape(C_IN, KH * KW * C_OUT)
        wp = np.ascontiguousarray(np.concatenate([wp, wp], axis=0)).astype(
            np.float16)
        in_maps.append({
            "x": x,
            "w": wp,
            "bias": np.ascontiguousarray(bias[e].reshape(C_OUT, 1)),
        })
    return in_maps


def _assemble(results):
    out = np.empty((E, B, C_OUT, HO, WO), dtype=np.float32)
    for c in range(N_CORES):
        e, half = divmod(c, 2)
        b0 = half * B_PER_CORE
        out[e, b0:b0 + B_PER_CORE] = results[c]["y"].astype(
            np.float32).reshape(B_PER_CORE, C_OUT, HO, WO)
    return out


def kernel(features, weight, bias):
    nc = _get_nc()
    in_maps = _make_in_maps(features, weight, bias)
    res = run_bass_kernel_spmd(nc, in_maps, core_ids=list(range(N_CORES)))
    return _assemble(res.results)


# revision 16
# speedup vs baseline: 1.0032x; 1.0032x over previous
"""Trainium2 Bass kernel for ensemble Conv2d (VALID, 3x3).

Problem: out[e,b,j,y,x] = sum_{i,kh,kw} features[e,b,i,y+kh,x+kw] * weight[e,i,j,kh,kw] + bias[e,j]
  features: (4, 32, 64, 64, 64) f32, weight: (4, 64, 128, 3, 3) f32, bias: (4, 128) f32
  output:   (4, 32, 128, 62, 62) f32

Sharding: E*B = 128 images over 8 cores -> each core handles one ensemble
member e = core//2 and 16 of its 32 images. No cross-core communication.

Per-core kernel: implicit-GEMM convolution. For each image, each 3x3 tap
(kh,kw) is one matmul contraction over C_in=64:
  psum[j, (y,x)] += W[:,j,kh,kw].T @ X[:, y+kh, x+kw]
fp16 matmuls (1 col/cycle, fp32 PSUM accumulation; rel err ~3e-4). Two
images are processed concurrently on the two PE-array row halves
(tile_position (0,0) and (64,0)) so the K=64 contraction doesn't waste half
the array; their weight copies live on the matching partition halves.

PSUM evacuation (bias add + f32->fp16 downcast) alternates between the
vector and gpsimd engines via tensor_scalar_add - no scalar activation, so
no ACT_TABLE_LOAD on the critical path. Output is stored fp16 (half the
HBM write traffic; host upcasts) in 4-row-block merged stores.
"""

import ml_dtypes
import numpy as np

import concourse.bass as bass
import concourse.mybir as mybir
import concourse.tile as tile
from concourse import bacc
from concourse.alu_op_type import AluOpType
from concourse.bass_utils import run_bass_kernel_spmd

E, B, C_IN, H, W = 4, 32, 64, 64, 64
C_OUT, KH, KW = 128, 3, 3
HO, WO = H - KH + 1, W - KW + 1  # 62, 62
N_CORES = 8
B_PER_CORE = (E * B) // N_CORES  # 16
PAIRS = B_PER_CORE // 2  # 8
ROW_BLOCK = 8  # output rows per matmul block: N = 8*62 = 496 <= 512 (one PSUM bank)
ROW_BLOCKS = [(r * ROW_BLOCK, min(ROW_BLOCK, HO - r * ROW_BLOCK))
              for r in range((HO + ROW_BLOCK - 1) // ROW_BLOCK)]
# merge groups of 4 row blocks into single output stores
STORE_GROUPS = [(0, 4), (4, 4)]  # (first block idx, n blocks)
F32 = mybir.dt.float32
F32R = mybir.dt.float32r
BF16 = mybir.dt.bfloat16
FP16 = mybir.dt.float16

MM_DT = FP16
OUT_DT = FP16

_CACHE: dict = {}


def _build():
    nc = bacc.Bacc("TRN2", target_bir_lowering=False, debug=False,
                   num_devices=N_CORES)
    x_d = nc.dram_tensor("x", [B_PER_CORE * C_IN, H * W], MM_DT,
                         kind="ExternalInput").ap()
    w_d = nc.dram_tensor("w", [128, KH * KW * C_OUT], MM_DT,
                         kind="ExternalInput").ap()
    b_d = nc.dram_tensor("bias", [C_OUT, 1], F32, kind="ExternalInput").ap()
    y_d = nc.dram_tensor("y", [B_PER_CORE * C_OUT, HO * WO], OUT_DT,
                         kind="ExternalOutput").ap()

    with tile.TileContext(nc) as tc:
        with (
            tc.tile_pool(name="wpool", bufs=1) as wpool,
            tc.tile_pool(name="xpool", bufs=6) as xpool,
            tc.tile_pool(name="opool", bufs=6) as opool,
            tc.tile_pool(name="psum", bufs=8, space=bass.MemorySpace.PSUM) as psum,
        ):
            # Ramp-critical loads on the two HWDGE rings (the gpsimd ring is
            # SWDGE - ~3us slower to first byte, so it carries no DMAs at
            # all): weights on scalar, the first 10 input rows of pair 0
            # (exactly what the first row-block needs) alone on sync so no
            # other packets share its queue.
            # tap 0's weight columns land first (33KB) so the first
            # LDWEIGHTS fires ~2.5us earlier than waiting for all of w
            w_sb = wpool.tile([128, KH * KW * C_OUT], MM_DT)
            nc.scalar.dma_start(w_sb[:, 0:C_OUT], w_d[:, 0:C_OUT])
            nc.scalar.dma_start(w_sb[:, C_OUT:], w_d[:, C_OUT:])
            bias_sb = wpool.tile([C_OUT, 1], F32)
            nc.scalar.dma_start(bias_sb[:, :], b_d[:, :])

            x_tiles = [None] * PAIRS
            C0 = (ROW_BLOCK + KH - 1) * W  # 640 cols = input rows 0-9

            def issue_x_dma(p):
                # image pair p: image 2p on partitions 0-63, 2p+1 on 64-127.
                # Column-chunks so sub-tile deps let each row-block start as
                # soon as its chunk lands.
                x_sb = xpool.tile([128, H * W], MM_DT, tag="x")
                bounds = [0, 1024, 2048, 3072, H * W]
                if p == 0:
                    nc.sync.dma_start(x_sb[:, 0:C0], x_d[0:128, 0:C0])
                    bounds = [C0, 1408, 2176, 3072, H * W]
                for c0, c1 in zip(bounds[:-1], bounds[1:]):
                    nc.scalar.dma_start(x_sb[:, c0:c1],
                                        x_d[p * 128:(p + 1) * 128, c0:c1])
                x_tiles[p] = x_sb

            issue_x_dma(0)
            issue_x_dma(1)

            issue_x_dma(2)

            for p in range(PAIRS):
                if p + 3 < PAIRS:
                    issue_x_dma(p + 3)
                xv = x_tiles[p].rearrange("p (r c) -> p r c", c=W)
                # one fp16 output tile per (half, store-group): 4 blocks merged
                o_sb = {}
                for h in (0, 1):
                    for g, (g0, gn) in enumerate(STORE_GROUPS):
                        ncols = sum(nr * WO for (R, nr) in ROW_BLOCKS[g0:g0 + gn])
                        o_sb[(h, g)] = opool.tile([C_OUT, ncols], OUT_DT,
                                                  tag="o", name=f"o{p}_{h}_{g}")
                for bi, (R, nr) in enumerate(ROW_BLOCKS):
                    n_free = nr * WO
                    ps = [psum.tile([C_OUT, n_free], F32, tag="ps",
                                    name=f"ps{p}_{R}_{h}")
                          for h in range(2)]
                    for t in range(KH * KW):
                        kh, kw = divmod(t, KW)
                        for h in (0, 1):
                            rhs = xv[64 * h:64 * h + 64,
                                     R + kh:R + kh + nr, kw:kw + WO]
                            lhsT = w_sb[64 * h:64 * h + 64,
                                        t * C_OUT:(t + 1) * C_OUT]
                            nc.tensor.matmul(
                                ps[h][:, :],
                                lhsT,
                                rhs,
                                start=(t == 0),
                                stop=(t == KH * KW - 1),
                                tile_position=(64 * h, 0),
                            )
                    g = 0 if bi < STORE_GROUPS[0][1] else 1
                    goff = (R - ROW_BLOCKS[STORE_GROUPS[g][0]][0]) * WO
                    for h in (0, 1):
                        # evacuate psum (bias add + downcast). Mid-stream:
                        # vector only, so the scalar ring stays pure-DMA and
                        # x prefetch triggers never queue behind psum-gated
                        # ACTs (that delays psum-bank reuse and stalls the
                        # matmul stream). Last pair: split vector/scalar so
                        # the final evacs run in parallel (gpsimd can't read
                        # PSUM) and the tail drains sooner.
                        if h == 1 and p == PAIRS - 1:
                            nc.scalar.activation(
                                o_sb[(h, g)][:, goff:goff + n_free],
                                ps[h][:, :],
                                mybir.ActivationFunctionType.Identity,
                                bias=bias_sb[:, :])
                        else:
                            nc.vector.tensor_scalar_add(
                                o_sb[(h, g)][:, goff:goff + n_free],
                                ps[h][:, :], bias_sb[:, :])
                for h in (0, 1):
                    b_img = p * 2 + h
                    for g, (g0, gn) in enumerate(STORE_GROUPS):
                        c0 = ROW_BLOCKS[g0][0] * WO
                        ncols = o_sb[(h, g)].shape[1]
                        if p < PAIRS - 1:
                            nc.sync.dma_start(
                                y_d[b_img * C_OUT:(b_img + 1) * C_OUT,
                                    c0:c0 + ncols],
                                o_sb[(h, g)][:, :])
                        else:
                            # last pair: per-block stores so the final store
                            # begins as soon as its own evac lands
                            off = 0
                            for (R, nr) in ROW_BLOCKS[g0:g0 + gn]:
                                n = nr * WO
                                nc.sync.dma_start(
                                    y_d[b_img * C_OUT:(b_img + 1) * C_OUT,
                                        c0 + off:c0 + off + n],
                                    o_sb[(h, g)][:, off:off + n])
                                off += n
    nc.compile()
    return nc


def _get_nc():
    if "nc" not in _CACHE:
        _CACHE["nc"] = _build()
    return _CACHE["nc"]


def _make_in_maps(features, weight, bias):
    features = np.asarray(features, dtype=np.float32)
    weight = np.asarray(weight, dtype=np.float32)
    bias = np.asarray(bias, dtype=np.float32)
    in_maps = []
    for c in range(N_CORES):
        e, half = divmod(c, 2)
        b0 = half * B_PER_CORE
        x = np.ascontiguousarray(features[e, b0:b0 + B_PER_CORE]).reshape(
            B_PER_CORE * C_IN, H * W).astype(np.float16)
        # w[i, (kh*KW+kw)*C_OUT + j] = weight[e, i, j, kh, kw]; duplicated on
        # partitions 64-127 for the upper-row-half matmuls.
        wp = weight[e].transpose(0, 2, 3, 1).reshape(C_IN, KH * KW * C_OUT)
        wp = np.ascontiguousarray(np.concatenate([wp, wp], axis=0)).astype(
            np.float16)
        in_maps.append({
            "x": x,
            "w": wp,
            "bias": np.ascontiguousarray(bias[e].reshape(C_OUT, 1)),
        })
    return in_maps


def _assemble(results):
    out = np.empty((E, B, C_OUT, HO, WO), dtype=np.float32)
    for c in range(N_CORES):
        e, half = divmod(c, 2)
        b0 = half * B_PER_CORE
        out[e, b0:b0 + B_PER_CORE] = results[c]["y"].astype(
            np.float32).reshape(B_PER_CORE, C_OUT, HO, WO)
    return out


def kernel(features, weight, bias):
    nc = _get_nc()
    in_maps = _make_in_maps(features, weight, bias)
    res = run_bass_kernel_spmd(nc, in_maps, core_ids=list(range(N_CORES)))
    return _assemble(res.results)
